# revision 3
# baseline (speedup 1.0000x reference)
"""Trainium2 Bass kernel for nn_FeatLUT (embedding_lookup -> global mean).

Same contract as kernel.py. Key change vs baseline: the one-hot generation
uses per-bin DVE tensor_scalar(is_equal) ops in a [part, bin, pix] layout
(bf16, SBUF, step-1 APs -> DVE 4x perf mode) instead of one wide 1x-mode
tensor_tensor, with a pair trick (one op writes bin i for both the a-side
and b-side via a strided AP) to halve the per-op SBUF-access overhead, and
a tail of bins offloaded to GPSIMD (Pool) which runs concurrently.

Index math per pixel: j = 289*x0 + 17*x1 + x2 in [0, 4913); split
j = DIV*a + b; hist[a, b] accumulated on the PE via per-pixel-column
outer-product matmuls; out = (hist_m @ lutm + hist_l @ lutl) finished with
mean/round/clamp on host across the 8 cores.
"""

import sys

sys.path.insert(0, "/opt/trn_rl_repo")

import numpy as np

N_CORES = 8
H = W = 2048
ROWS = H // N_CORES  # 256
NFEAT = 20
MAGIC = 12582912.0  # 1.5 * 2^23

# ---- tunables -------------------------------------------------------------
DIV = 71          # j = DIV*a + b
A_BINS = 70       # ceil(4913 / DIV)
B_BINS = 71       # == DIV
FD = 320          # pixel-columns per one-hot chunk
POOL_PAIRS = 12   # pair-bins (from the top of the pair range) done on Pool
POOL_SINGLES = True  # Pool also does the unpaired tail bins
ACT_PAIRS = 8     # pair-bins done on the Activation engine (Abs+Relu)
T_ON_ACT = True   # compute t = j/div - off on ACT (Copy w/ scale+bias)
# ---------------------------------------------------------------------------

TOT_BINS = A_BINS + B_BINS
N_PAIR = min(A_BINS, B_BINS)

LAST_EXEC_NS = None
LAST_TRACE = None
TRACE = False
_CACHED = None


def _build(div=DIV, a_bins=A_BINS, b_bins=B_BINS, fd=FD, pool_pairs=POOL_PAIRS,
           pool_singles=POOL_SINGLES, act_pairs=ACT_PAIRS, t_on_act=T_ON_ACT,
           n_chunks=None, x_bufs=2):
    from contextlib import ExitStack

    import concourse.bacc as bacc
    import concourse.bass as bass
    import concourse.mybir as mybir
    import concourse.tile as tile

    f32 = mybir.dt.float32
    bf16 = mybir.dt.bfloat16
    i16 = mybir.dt.int16
    f16 = mybir.dt.float16
    A = mybir.AluOpType

    tot_bins = a_bins + b_bins
    n_pair = min(a_bins, b_bins)
    n_single = tot_bins - 2 * n_pair  # unpaired tail (on the larger side)
    a_larger = a_bins > b_bins
    # rounding offset for a = round(j/div - OFF): fractions k/div keep a
    # safe margin from .5 boundaries for div in {64, 71} (verified in tests)
    off = 0.4965

    nc = bacc.Bacc("TRN2", target_bir_lowering=False, debug=False)
    xin = nc.dram_tensor("xin", [3, ROWS, W], f32, kind="ExternalInput")
    xs = nc.dram_tensor("xs", [3, ROWS, W], f32, kind="ExternalInput")
    tmsb = nc.dram_tensor("tmsb", [a_bins, b_bins * NFEAT], f16, kind="ExternalInput")
    tlsb = nc.dram_tensor("tlsb", [a_bins, b_bins * NFEAT], f16, kind="ExternalInput")
    out = nc.dram_tensor("out", [1, NFEAT], f32, kind="ExternalOutput")

    n_rb = ROWS // 128
    if n_chunks is not None:
        # near-even split of W into n_chunks even-sized column chunks
        base = (W // n_chunks) & ~1
        sizes = [base] * n_chunks
        extra = W - base * n_chunks
        i = 0
        while extra > 0:
            sizes[i] += 2
            extra -= 2
            i = (i + 1) % n_chunks
        col_chunks = []
        c0 = 0
        for s in sizes:
            col_chunks.append((c0, s))
            c0 += s
        fd = max(sizes)
    else:
        # column chunks: full-width fd chunks plus one remainder if needed
        col_chunks = []
        c0 = 0
        while c0 < W:
            cw = min(fd, W - c0)
            col_chunks.append((c0, cw))
            c0 += cw

    with tile.TileContext(nc) as tc:
        with ExitStack() as octx:
            singles = octx.enter_context(tc.tile_pool(name="singles", bufs=1))
            psum = octx.enter_context(tc.tile_pool(name="psum", bufs=1, space="PSUM"))

            n_pool_sing = n_single if pool_singles else 0
            cb_act = None
            if act_pairs > 0:
                cb_act = singles.tile([128, n_pair], f32)
                nc.gpsimd.iota(
                    cb_act,
                    pattern=[[1, n_pair]],
                    base=0,
                    channel_multiplier=0,
                    allow_small_or_imprecise_dtypes=True,
                )

            hist_m = psum.tile([a_bins, b_bins], f32)
            hist_l = psum.tile([a_bins, b_bins], f32)
            acc = psum.tile([1, NFEAT], f32)

            # image-m contraction inputs live in `singles` so the m-side
            # LUT contraction can overlap image-l's main loop
            lut_m = singles.tile([a_bins, b_bins * NFEAT], f16)
            hist_m_sb = singles.tile([a_bins, b_bins], f16)
            lut_m_loaded = [False]

            ctx = octx.enter_context(ExitStack())
            xpool = ctx.enter_context(tc.tile_pool(name="xpool", bufs=x_bufs))
            prep1 = ctx.enter_context(tc.tile_pool(name="prep1", bufs=1))
            prep = ctx.enter_context(tc.tile_pool(name="prep", bufs=2))
            ohp = ctx.enter_context(tc.tile_pool(name="ohp", bufs=2))

            def split_tail(chunks):
                # replace the trailing chunk with a descending pair so the
                # final PE drain after the last one-hot is short
                c0, cw = chunks[-1]
                if cw < 256:
                    return chunks
                w1 = ((cw * 5 // 8) + 1) & ~1
                return chunks[:-1] + [(c0, w1), (c0 + w1, cw - w1)]

            def do_image(xdram, hist, first=False, last=False):
                mm = 0
                total_mm = n_rb * W
                for rb in range(n_rb):
                    rs = slice(rb * 128, (rb + 1) * 128)
                    chunks = list(col_chunks)
                    if first and rb == 0:
                        # ascending head for a faster ramp-in
                        head = split_tail([chunks[0]])
                        chunks = head[::-1] + chunks[1:]
                    if last and rb == n_rb - 1:
                        chunks = split_tail(chunks)
                        chunks = chunks[:-1] + split_tail([chunks[-1]])
                    for c0, cw in chunks:
                        cs = slice(c0, c0 + cw)
                        x0 = xpool.tile([128, fd], f32, tag="x0")
                        x1 = xpool.tile([128, fd], f32, tag="x1")
                        x2 = xpool.tile([128, fd], f32, tag="x2")
                        nc.sync.dma_start(out=x0[:, :cw], in_=xdram[0, rs, cs])
                        nc.sync.dma_start(out=x1[:, :cw], in_=xdram[1, rs, cs])
                        nc.sync.dma_start(out=x2[:, :cw], in_=xdram[2, rs, cs])
                        if not lut_m_loaded[0]:
                            lut_m_loaded[0] = True
                            nc.sync.dma_start(out=lut_m, in_=tmsb[:, :])

                        u = prep.tile([128, fd], f32, tag="u")
                        nc.vector.scalar_tensor_tensor(
                            out=u[:, :cw], in0=x0[:, :cw], scalar=17.0,
                            in1=x1[:, :cw], op0=A.mult, op1=A.add,
                        )
                        j = prep.tile([128, fd], f32, tag="j")
                        nc.vector.scalar_tensor_tensor(
                            out=j[:, :cw], in0=u[:, :cw], scalar=17.0,
                            in1=x2[:, :cw], op0=A.mult, op1=A.add,
                        )
                        t = prep.tile([128, fd], f32, tag="u")
                        if t_on_act:
                            nc.scalar.activation(
                                out=t[:, :cw],
                                in_=j[:, :cw],
                                func=mybir.ActivationFunctionType.Copy,
                                bias=-off,
                                scale=1.0 / div,
                            )
                        else:
                            nc.vector.tensor_scalar(
                                out=t[:, :cw],
                                in0=j[:, :cw],
                                scalar1=1.0 / div,
                                scalar2=off,
                                op0=A.mult,
                                op1=A.subtract,
                            )
                        # ab[:, 0, :] = a (rounded), ab[:, 1, :] = b = j - div*a
                        ab = prep.tile([128, 2, fd], bf16, tag="ab")
                        arow = bass.AP(
                            tensor=ab.tensor, offset=ab.offset, ap=[ab.ap[0], [1, cw]]
                        )
                        nc.vector.tensor_scalar(
                            out=arow,
                            in0=t[:, :cw],
                            scalar1=MAGIC,
                            scalar2=MAGIC,
                            op0=A.add,
                            op1=A.subtract,
                        )
                        brow = bass.AP(
                            tensor=ab.tensor,
                            offset=ab.offset + fd,
                            ap=[ab.ap[0], [1, cw]],
                        )
                        nc.vector.scalar_tensor_tensor(
                            out=brow,
                            in0=arow,
                            scalar=-float(div),
                            in1=j[:, :cw],
                            op0=A.mult,
                            op1=A.add,
                        )

                        # one-hot tile: rows [0, a_bins) = a-side, rows
                        # [a_bins, tot_bins) = b-side
                        oh = ohp.tile([128, tot_bins, fd], bf16, tag="oh")

                        # paired DVE ops: bin i of both sides in one op
                        n_dve_pair = n_pair - pool_pairs - act_pairs
                        for i in range(n_dve_pair):
                            o2 = bass.AP(
                                tensor=oh.tensor,
                                offset=oh.offset + i * fd,
                                ap=[oh.ap[0], [a_bins * fd, 2], [1, cw]],
                            )
                            i2 = bass.AP(
                                tensor=ab.tensor,
                                offset=ab.offset,
                                ap=[ab.ap[0], [fd, 2], [1, cw]],
                            )
                            nc.vector.tensor_scalar(
                                out=o2, in0=i2, scalar1=float(i), scalar2=None,
                                op0=A.is_equal,
                            )

                        # ACT pairs: oh = Relu(1 - Abs(ab - i)), exact for ints
                        if act_pairs > 0:
                            FT = mybir.ActivationFunctionType
                            # ping-pong scratch: Abs(i+1) doesn't WAR-chain on
                            # Relu(i)'s read of the other slice
                            s_act = prep1.tile([128, 2, 2, fd], bf16, tag="acts")
                            i2a = bass.AP(
                                tensor=ab.tensor,
                                offset=ab.offset,
                                ap=[ab.ap[0], [fd, 2], [1, cw]],
                            )
                            for ii, i in enumerate(
                                range(n_dve_pair, n_dve_pair + act_pairs)
                            ):
                                sl = bass.AP(
                                    tensor=s_act.tensor,
                                    offset=s_act.offset + (ii % 2) * 2 * fd,
                                    ap=[s_act.ap[0], [fd, 2], [1, cw]],
                                )
                                o2 = bass.AP(
                                    tensor=oh.tensor,
                                    offset=oh.offset + i * fd,
                                    ap=[oh.ap[0], [a_bins * fd, 2], [1, cw]],
                                )
                                # s = |i - ab|  (scale=-1, bias=i from iota tile)
                                nc.scalar.activation(
                                    out=sl, in_=i2a, func=FT.Abs,
                                    bias=cb_act[:, i : i + 1], scale=-1.0,
                                )
                                # oh = relu(1 - s)
                                nc.scalar.activation(
                                    out=o2, in_=sl, func=FT.Relu, bias=1.0, scale=-1.0
                                )

                        # unpaired tail (larger side), values n_pair..n_pair+n_single
                        if n_single > 0 and not pool_singles:
                            srow = 0 if a_larger else fd  # source row in ab
                            obase = (n_pair if a_larger else a_bins + n_pair) * fd
                            for k in range(n_single):
                                o1 = bass.AP(
                                    tensor=oh.tensor,
                                    offset=oh.offset + obase + k * fd,
                                    ap=[oh.ap[0], [1, cw]],
                                )
                                i1 = bass.AP(
                                    tensor=ab.tensor,
                                    offset=ab.offset + srow,
                                    ap=[ab.ap[0], [1, cw]],
                                )
                                nc.vector.tensor_scalar(
                                    out=o1, in0=i1, scalar1=float(n_pair + k),
                                    scalar2=None, op0=A.is_equal,
                                )

                        # Pool: tail of the pair range as per-pair tensor_scalar
                        # (Pool supports TensorScalar but not TensorTensor)
                        for i in range(n_pair - pool_pairs, n_pair):
                            o2 = bass.AP(
                                tensor=oh.tensor,
                                offset=oh.offset + i * fd,
                                ap=[oh.ap[0], [a_bins * fd, 2], [1, cw]],
                            )
                            i2 = bass.AP(
                                tensor=ab.tensor,
                                offset=ab.offset,
                                ap=[ab.ap[0], [fd, 2], [1, cw]],
                            )
                            nc.gpsimd.tensor_scalar(
                                out=o2, in0=i2, scalar1=float(i), scalar2=None,
                                op0=A.is_equal,
                            )
                        if n_pool_sing > 0:
                            srow = 0 if a_larger else fd
                            row0 = n_pair if a_larger else a_bins + n_pair
                            for k in range(n_pool_sing):
                                o1 = bass.AP(
                                    tensor=oh.tensor,
                                    offset=oh.offset + (row0 + k) * fd,
                                    ap=[oh.ap[0], [1, cw]],
                                )
                                i1 = bass.AP(
                                    tensor=ab.tensor,
                                    offset=ab.offset + srow,
                                    ap=[ab.ap[0], [1, cw]],
                                )
                                nc.gpsimd.tensor_scalar(
                                    out=o1, in0=i1, scalar1=float(n_pair + k),
                                    scalar2=None, op0=A.is_equal,
                                )

                        # hist matmuls: one per pixel column
                        for x in range(cw):
                            lhs = bass.AP(
                                tensor=oh.tensor,
                                offset=oh.offset + x,
                                ap=[oh.ap[0], [fd, a_bins]],
                            )
                            rhs = bass.AP(
                                tensor=oh.tensor,
                                offset=oh.offset + a_bins * fd + x,
                                ap=[oh.ap[0], [fd, b_bins]],
                            )
                            nc.tensor.matmul(
                                hist[:, :],
                                lhs,
                                rhs,
                                start=(mm == 0),
                                stop=(mm == total_mm - 1),
                            )
                            mm += 1

            def contract(hist_sb, lut, start, stop):
                for bb in range(b_bins):
                    nc.tensor.matmul(
                        acc[:, :],
                        hist_sb[:, bb : bb + 1],
                        lut[:, bb * NFEAT : (bb + 1) * NFEAT],
                        start=(start and bb == 0),
                        stop=(stop and bb == b_bins - 1),
                    )

            do_image(xin, hist_m)
            do_image(xs, hist_l, last=True)

            # main-loop pools released; tail pool reuses their SBUF
            ctx.close()
            tail = octx.enter_context(tc.tile_pool(name="tail", bufs=1))
            lut_l = tail.tile([a_bins, b_bins * NFEAT], f16)
            nc.sync.dma_start(out=lut_l, in_=tlsb[:, :])
            nc.vector.tensor_copy(hist_m_sb, hist_m)
            contract(hist_m_sb, lut_m, start=True, stop=False)
            hist_l_sb = tail.tile([a_bins, b_bins], f16)
            nc.vector.tensor_copy(hist_l_sb, hist_l)
            contract(hist_l_sb, lut_l, start=False, stop=True)

            out_sb = tail.tile([1, NFEAT], f32)
            nc.vector.tensor_copy(out_sb, acc)
            nc.sync.dma_start(out=out[:, :], in_=out_sb)

    nc.compile()
    return nc


def _prep_table(feat, div=DIV, a_bins=A_BINS, b_bins=B_BINS):
    """[78608,20,1,1] int8 -> [a_bins, b_bins*20] f32 (LUT16 in a-major layout).

    Only every 16th LUT row is reachable (channel weights share factor 16):
    LUT16 = LUT[::16] with 4913 rows; entry (a, b) maps to LUT16[div*a + b].
    """
    t = np.asarray(feat).reshape(78608, NFEAT)[::16].astype(np.float16)  # [4913,20]
    pad = np.zeros((a_bins * div, NFEAT), np.float16)
    pad[: t.shape[0]] = t
    if div != b_bins:
        # b only takes values < div, but the tile is [a_bins, b_bins]; pad
        pad = pad.reshape(a_bins, div, NFEAT)
        full = np.zeros((a_bins, b_bins, NFEAT), np.float16)
        full[:, :div] = pad
        return np.ascontiguousarray(full.reshape(a_bins, b_bins * NFEAT))
    return np.ascontiguousarray(pad.reshape(a_bins, b_bins * NFEAT))


def kernel(x_in, x_s, feature_msb, feature_lsb):
    global LAST_EXEC_NS, LAST_TRACE, _CACHED
    from concourse import bass_utils

    if _CACHED is None:
        _CACHED = _build(n_chunks=6, x_bufs=1)
    nc = _CACHED

    x_in = np.ascontiguousarray(np.asarray(x_in, dtype=np.float32).reshape(3, H, W))
    x_s = np.ascontiguousarray(np.asarray(x_s, dtype=np.float32).reshape(3, H, W))
    tm = _prep_table(feature_msb)
    tl = _prep_table(feature_lsb)

    in_maps = []
    for c in range(N_CORES):
        rs = slice(c * ROWS, (c + 1) * ROWS)
        in_maps.append(
            {
                "xin": np.ascontiguousarray(x_in[:, rs, :]),
                "xs": np.ascontiguousarray(x_s[:, rs, :]),
                "tmsb": tm,
                "tlsb": tl,
            }
        )

    try:
        res = bass_utils.run_bass_kernel_spmd(
            nc, in_maps, core_ids=list(range(N_CORES)), trace=TRACE
        )
    except Exception:
        # transient device errors have been observed on this fabric; one
        # retry clears them
        res = bass_utils.run_bass_kernel_spmd(
            nc, in_maps, core_ids=list(range(N_CORES)), trace=TRACE
        )
    LAST_EXEC_NS = res.exec_time_ns
    LAST_TRACE = res.instructions_and_trace

    s = np.zeros(NFEAT, np.float64)
    for rr in res.results:
        s += rr["out"].astype(np.float64).reshape(NFEAT)
    mean = s / float(H * W)
    q = np.clip(np.round(mean * 4.0) / 4.0, -32.0, 31.75)
    return q.reshape(1, NFEAT, 1, 1).astype(np.float32)
